# revision 18
# baseline (speedup 1.0000x reference)
"""Slot-attention kernel (nn_AttentionModule_39084202394083) for 8x TRN2 cores.

Strategy
--------
Data-parallel over batch B=64 across 8 cores (8 batches/core). Per core:

  x_ln = unit-LN(inputs)  (g_in/b_in folded into downstream weights)
  kept resident in SBUF twice, in fp8e4m3:
    Y_nat [pos, d]  (moving operand of the updates matmul)
    Y_T   [d, pos]  (stationary operand of the dots matmul)

  The k/v projections never materialize:
    dots  = q @ k^T   == u @ (wq~)^T  with wq~ = diag(g_sl) wq wk_g^T / sqrt(D)
    updates = attn @ v == ((attn @ u) @ wv_g) / den  (+ bias terms)
  so the two big [B*N,D]x[D,D] matmuls disappear; only tiny [64,256]
  slot-side matmuls remain, all folded constants computed on device.

  Softmax over slots is a free-axis reduction in the dots^T [pos, slot]
  layout; the +EPS renormalization term is a ones-column matmul sharing
  the attn stationary weights.

Precision: fp8 storage for x_ln / attn / q (softmax logits are O(+-2),
PSUM accumulation is fp32), bf16 for weights & slot-phase operands,
fp32 for LN stats / softmax / GRU elementwise. Tolerance is 2e-2.
"""

import sys
import numpy as np

sys.path.insert(0, "/opt/trn_rl_repo")

from contextlib import ExitStack  # noqa: E402

import concourse.bass as bass  # noqa: E402
import concourse.tile as tile  # noqa: E402
from concourse import bacc, mybir  # noqa: E402
from concourse.bass_utils import run_bass_kernel_spmd  # noqa: E402
from concourse.masks import make_identity  # noqa: E402


def _install_ntff_hook():
    """Provide antenv.axon_hooks (absent in this image) so trace=True can
    capture NTFF profiles through libaxon_pjrt's nrt-profile side channel."""
    import types

    if "antenv.axon_hooks" in sys.modules:
        return
    mod = types.ModuleType("antenv.axon_hooks")
    holder = [None]
    mod.set_axon_ntff_profile_hook = lambda h: holder.__setitem__(0, h)
    mod.get_axon_ntff_profile_hook = lambda: holder[0]
    sys.modules["antenv.axon_hooks"] = mod
    try:
        import antenv

        antenv.axon_hooks = mod
    except Exception:
        pass
    try:
        if "/root/.axon_site" not in sys.path:
            sys.path.insert(0, "/root/.axon_site")
        from trn_agent_boot.trn_boot import _ntff_profile_via_ctypes

        h = _ntff_profile_via_ctypes("/opt/axon/libaxon_pjrt.so")
        if h is not None:
            mod.set_axon_ntff_profile_hook(h)
    except Exception:
        pass


_install_ntff_hook()

F32 = mybir.dt.float32
BF16 = mybir.dt.bfloat16
FP8 = mybir.dt.float8e4

NCORES = 8
B = 64
N = 4096
D = 256
S = 8
H = 1024
ITERS = 3
EPS = 1e-8
LN_EPS = 1e-5

BL = B // NCORES          # batches per core
R = BL * S                # slot rows per core (row = b_local*S + s)
NT = N // 128             # 128-pos tiles per batch
DC = D // 128             # 128-chunks of D
HC = H // 128             # 128-chunks of H
SCALE = float(D) ** -0.5

Alu = mybir.AluOpType
Act = mybir.ActivationFunctionType


def _col(ap_1d, p=128):
    """[K] dram vector -> [p, K//p] column-chunk view (i = c*p + part)."""
    return ap_1d.rearrange("(c p) -> p c", p=p)


def _bcast_free(ap, n):
    """Append a stride-0 innermost axis of size n to an AP."""
    return bass.AP(tensor=ap.tensor, offset=ap.offset, ap=list(ap.ap) + [[0, n]])


def build(nl=N, bl=BL, iters=ITERS, dbg=False):
    nt = nl // 128
    r = bl * S
    nc = bacc.Bacc("TRN2", target_bir_lowering=False, debug=False)

    x_d = nc.dram_tensor("x", [bl, nl, D], F32, kind="ExternalInput").ap()
    slots_d = nc.dram_tensor("slots0", [r, D], F32, kind="ExternalInput").ap()
    wq_d = nc.dram_tensor("wq", [D, D], F32, kind="ExternalInput").ap()
    bq_d = nc.dram_tensor("bq", [D], F32, kind="ExternalInput").ap()
    wk_d = nc.dram_tensor("wk", [D, D], F32, kind="ExternalInput").ap()
    bk_d = nc.dram_tensor("bk", [D], F32, kind="ExternalInput").ap()
    wv_d = nc.dram_tensor("wv", [D, D], F32, kind="ExternalInput").ap()
    bv_d = nc.dram_tensor("bv", [D], F32, kind="ExternalInput").ap()
    wih_d = nc.dram_tensor("w_ih", [D, 3 * D], F32, kind="ExternalInput").ap()
    bih_d = nc.dram_tensor("b_ih", [3 * D], F32, kind="ExternalInput").ap()
    whh_d = nc.dram_tensor("w_hh", [D, 3 * D], F32, kind="ExternalInput").ap()
    bhh_d = nc.dram_tensor("b_hh", [3 * D], F32, kind="ExternalInput").ap()
    w1_d = nc.dram_tensor("mlp_w1", [D, H], F32, kind="ExternalInput").ap()
    b1_d = nc.dram_tensor("mlp_b1", [H], F32, kind="ExternalInput").ap()
    w2_d = nc.dram_tensor("mlp_w2", [H, D], F32, kind="ExternalInput").ap()
    b2_d = nc.dram_tensor("mlp_b2", [D], F32, kind="ExternalInput").ap()
    gin_d = nc.dram_tensor("g_in", [D], F32, kind="ExternalInput").ap()
    bin_d = nc.dram_tensor("b_in", [D], F32, kind="ExternalInput").ap()
    gsl_d = nc.dram_tensor("g_sl", [D], F32, kind="ExternalInput").ap()
    bsl_d = nc.dram_tensor("b_sl", [D], F32, kind="ExternalInput").ap()
    gff_d = nc.dram_tensor("g_ff", [D], F32, kind="ExternalInput").ap()
    bff_d = nc.dram_tensor("b_ff", [D], F32, kind="ExternalInput").ap()
    out_d = nc.dram_tensor("out", [r, D], F32, kind="ExternalOutput").ap()

    with tile.TileContext(nc) as tc, ExitStack() as ctx:
        res = ctx.enter_context(tc.tile_pool(name="res", bufs=1))
        prep = ctx.enter_context(tc.tile_pool(name="prep", bufs=1))
        pio = ctx.enter_context(tc.tile_pool(name="pio", bufs=3))
        pln = ctx.enter_context(tc.tile_pool(name="pln", bufs=4))
        psm = ctx.enter_context(tc.tile_pool(name="psm", bufs=2))
        psl = ctx.enter_context(tc.tile_pool(name="psl", bufs=1))
        pp_dots = ctx.enter_context(tc.tile_pool(name="ppd", bufs=3, space="PSUM"))
        pp_ay = ctx.enter_context(tc.tile_pool(name="ppa", bufs=1, space="PSUM"))
        pp_w = ctx.enter_context(tc.tile_pool(name="ppw", bufs=4, space="PSUM"))

        # ---------------- constants / identities ----------------
        id_bf = prep.tile([128, 128], BF16)
        make_identity(nc, id_bf)
        id_f32 = prep.tile([128, 128], F32)
        make_identity(nc, id_f32)
        ones_r64_bf = prep.tile([1, r], BF16)
        nc.vector.memset(ones_r64_bf, 1.0)
        ones_r128_f8 = prep.tile([1, 128], BF16)
        nc.vector.memset(ones_r128_f8, 1.0)
        lneps_t = prep.tile([128, 1], F32)
        nc.vector.memset(lneps_t, LN_EPS)

        # ---------------- load & fold weights ----------------
        _ldn = [0]

        def load_f32(dram, shape, pool=prep, tag=None):
            _ldn[0] += 1
            t = pool.tile(shape, F32, tag=tag or f"ld{_ldn[0]}")
            nc.gpsimd.dma_start(t[:], dram)
            return t

        # column vectors [128, DC]
        gin_c = load_f32(_col(gin_d), [128, DC])
        bin_c = load_f32(_col(bin_d), [128, DC])
        gsl_c = load_f32(_col(gsl_d), [128, DC])
        bsl_c = load_f32(_col(bsl_d), [128, DC])
        gff_c = load_f32(_col(gff_d), [128, DC])
        bff_c = load_f32(_col(bff_d), [128, DC])
        bq_c = load_f32(_col(bq_d), [128, DC])
        bk_c = load_f32(_col(bk_d), [128, DC])
        bv_c = load_f32(_col(bv_d), [128, DC])
        b1_c = load_f32(_col(b1_d), [128, HC])

        wq_f = load_f32(wq_d.rearrange("(c p) j -> p c j", p=128), [128, DC, D])
        wk_f = load_f32(wk_d.rearrange("(c p) j -> p c j", p=128), [128, DC, D])
        wv_f = load_f32(wv_d.rearrange("(c p) j -> p c j", p=128), [128, DC, D])

        # wk_g = diag(g_in) wk ; wv_g = diag(g_in) wv
        wkg_f = prep.tile([128, DC, D], F32)
        wvg_bf = res.tile([128, DC, D], BF16)
        for c in range(DC):
            nc.vector.tensor_scalar_mul(wkg_f[:, c, :], wk_f[:, c, :], gin_c[:, c : c + 1])
            nc.vector.tensor_scalar_mul(wvg_bf[:, c, :], wv_f[:, c, :], gin_c[:, c : c + 1])

        # transpose wq, wk_g  ->  [o_part, oc, i]  (f32)
        wqT_f = prep.tile([128, DC, D], F32)
        wkgT_f = prep.tile([128, DC, D], F32)
        for src, dst in ((wq_f, wqT_f), (wkg_f, wkgT_f)):
            for ci in range(DC):
                for cj in range(DC):
                    tp = pp_w.tile([128, 128], F32, tag="w")
                    nc.tensor.transpose(tp, src[:, ci, cj * 128 : (cj + 1) * 128], id_f32)
                    nc.vector.tensor_copy(dst[:, cj, ci * 128 : (ci + 1) * 128], tp)

        # Wd = wq @ wk_g^T  [i, j];  Wd_s = diag(g_sl) Wd * SCALE (bf16)
        wd_raw = prep.tile([128, DC, D], F32)
        wd_s = res.tile([128, DC, D], BF16)
        for ti in range(DC):
            ps = pp_w.tile([128, D], F32, tag="w2")
            for co in range(DC):
                nc.tensor.matmul(
                    ps, wqT_f[:, co, ti * 128 : (ti + 1) * 128], wkgT_f[:, co, :],
                    start=(co == 0), stop=(co == DC - 1),
                )
            nc.vector.tensor_copy(wd_raw[:, ti, :], ps)
            nc.vector.tensor_scalar(
                wd_s[:, ti, :], ps, gsl_c[:, ti : ti + 1], SCALE, op0=Alu.mult, op1=Alu.mult
            )

        # qconst_j = (b_sl @ Wd + bq @ wk_g^T)_j * SCALE   [128, DC] f32 col
        qconst_c = prep.tile([128, DC], F32)
        for tj in range(DC):
            ps = pp_w.tile([128, 1], F32, tag="wc")
            for ci in range(DC):
                nc.tensor.matmul(
                    ps, wd_raw[:, ci, tj * 128 : (tj + 1) * 128], bsl_c[:, ci : ci + 1],
                    start=(ci == 0), stop=False,
                )
            for co in range(DC):
                nc.tensor.matmul(
                    ps, wkgT_f[:, co, tj * 128 : (tj + 1) * 128], bq_c[:, co : co + 1],
                    start=False, stop=(co == DC - 1),
                )
            nc.vector.tensor_scalar(
                qconst_c[:, tj : tj + 1], ps, SCALE, None, op0=Alu.mult
            )

        # bk' = b_in @ wk + bk  [o] col f32
        bkp_c = prep.tile([128, DC], F32)
        for to in range(DC):
            ps = pp_w.tile([128, 1], F32, tag="wc")
            for ci in range(DC):
                nc.tensor.matmul(
                    ps, wk_f[:, ci, to * 128 : (to + 1) * 128], bin_c[:, ci : ci + 1],
                    start=(ci == 0), stop=(ci == DC - 1),
                )
            nc.vector.tensor_add(bkp_c[:, to : to + 1], ps, bk_c[:, to : to + 1])

        # wqbk = wq @ bk'  [i] ; raw f32 and g_sl*SCALE-scaled bf16
        wqbk_raw = prep.tile([128, DC], F32)
        wqbk_s = prep.tile([128, DC], BF16)
        for ti in range(DC):
            ps = pp_w.tile([128, 1], F32, tag="wc")
            for co in range(DC):
                nc.tensor.matmul(
                    ps, wqT_f[:, co, ti * 128 : (ti + 1) * 128], bkp_c[:, co : co + 1],
                    start=(co == 0), stop=(co == DC - 1),
                )
            nc.vector.tensor_copy(wqbk_raw[:, ti : ti + 1], ps)
            nc.vector.tensor_scalar(
                wqbk_s[:, ti : ti + 1], ps, gsl_c[:, ti : ti + 1], SCALE,
                op0=Alu.mult, op1=Alu.mult,
            )

        # cst_scalar = (b_sl @ wqbk + bq @ bk') * SCALE   [1,1] bf16
        cstc_bf = prep.tile([1, 1], BF16)
        ps = pp_w.tile([1, 1], F32, tag="wc1")
        for ci in range(DC):
            nc.tensor.matmul(
                ps, wqbk_raw[:, ci : ci + 1], bsl_c[:, ci : ci + 1],
                start=(ci == 0), stop=False,
            )
        for co in range(DC):
            nc.tensor.matmul(
                ps, bkp_c[:, co : co + 1], bq_c[:, co : co + 1],
                start=False, stop=(co == DC - 1),
            )
        nc.vector.tensor_scalar(cstc_bf, ps, SCALE, None, op0=Alu.mult)

        # bv'' = b_in @ wv + bv  [o] col f32
        bvpp_c = prep.tile([128, DC], F32)
        for to in range(DC):
            ps = pp_w.tile([128, 1], F32, tag="wc")
            for ci in range(DC):
                nc.tensor.matmul(
                    ps, wv_f[:, ci, to * 128 : (to + 1) * 128], bin_c[:, ci : ci + 1],
                    start=(ci == 0), stop=(ci == DC - 1),
                )
            nc.vector.tensor_add(bvpp_c[:, to : to + 1], ps, bv_c[:, to : to + 1])

        # GRU / MLP weights
        wih_f = load_f32(wih_d.rearrange("(c p) j -> p c j", p=128), [128, DC, 3 * D], tag="wstage")
        wih_bf = res.tile([128, DC, 3 * D], BF16)
        for c in range(DC):
            nc.vector.tensor_copy(wih_bf[:, c, :], wih_f[:, c, :])
        whh_f = load_f32(whh_d.rearrange("(c p) j -> p c j", p=128), [128, DC, 3 * D], tag="wstage")
        whh_bf = res.tile([128, DC, 3 * D], BF16)
        for c in range(DC):
            nc.vector.tensor_copy(whh_bf[:, c, :], whh_f[:, c, :])

        w1_f = load_f32(w1_d.rearrange("(c p) j -> p c j", p=128), [128, DC, H], tag="wstage")
        w1g_bf = res.tile([128, DC, H], BF16)
        for c in range(DC):
            nc.vector.tensor_scalar_mul(w1g_bf[:, c, :], w1_f[:, c, :], gff_c[:, c : c + 1])
        # mlp bias col: (b_ff @ mlp_w1 + mlp_b1) [o] col f32 [128, HC]
        mlpb_c = prep.tile([128, HC], F32)
        for to in range(HC):
            ps = pp_w.tile([128, 1], F32, tag="wc")
            for ci in range(DC):
                nc.tensor.matmul(
                    ps, w1_f[:, ci, to * 128 : (to + 1) * 128], bff_c[:, ci : ci + 1],
                    start=(ci == 0), stop=(ci == DC - 1),
                )
            nc.vector.tensor_add(mlpb_c[:, to : to + 1], ps, b1_c[:, to : to + 1])

        w2_f = load_f32(w2_d.rearrange("(c p) j -> p c j", p=128), [128, HC, D], tag="wstage")
        w2_bf = res.tile([128, HC, D], BF16)
        for c in range(HC):
            nc.vector.tensor_copy(w2_bf[:, c, :], w2_f[:, c, :])

        # bias rows
        bih_row = prep.tile([1, 3 * D], BF16)
        tmp_row = prep.tile([1, 3 * D], F32, tag="rowstage")
        nc.gpsimd.dma_start(tmp_row[:], bih_d.rearrange("(o k) -> o k", o=1))
        nc.vector.tensor_copy(bih_row, tmp_row)
        bhh_row = prep.tile([1, 3 * D], BF16)
        tmp_row2 = prep.tile([1, 3 * D], F32, tag="rowstage")
        nc.gpsimd.dma_start(tmp_row2[:], bhh_d.rearrange("(o k) -> o k", o=1))
        nc.vector.tensor_copy(bhh_row, tmp_row2)
        b2_row = prep.tile([1, D], BF16)
        tmp_row3 = prep.tile([1, D], F32, tag="rowstage")
        nc.gpsimd.dma_start(tmp_row3[:], b2_d.rearrange("(o k) -> o k", o=1))
        nc.vector.tensor_copy(b2_row, tmp_row3)

        # initial slots
        slots = psl.tile([r, D], F32, tag="slots")
        nc.gpsimd.dma_start(slots[:], slots_d)

        pdbg = ctx.enter_context(tc.tile_pool(name="pdbg", bufs=1)) if dbg else None

        def tap(name, ap, shape):
            if not dbg:
                return
            t = pdbg.tile(list(shape), F32, tag=f"tap_{name}")
            nc.vector.tensor_copy(t[:], ap)
            od = nc.dram_tensor(f"dbg_{name}", list(shape), F32, kind="ExternalOutput").ap()
            nc.gpsimd.dma_start(od, t[:])

        # ---------------- per-iteration helpers ----------------
        def q_compute(slots_ap, it):
            st6 = psl.tile([r, 6], F32, tag="sst")
            nc.vector.bn_stats(st6[:], slots_ap[:])
            mv = psl.tile([r, 2], F32, tag="smv")
            nc.vector.bn_aggr(mv[:], st6[:])
            sd = psl.tile([r, 1], F32, tag="ssd")
            nc.scalar.activation(sd[:], mv[:, 1:2], Act.Sqrt, bias=lneps_t[:r])
            rsl = psl.tile([r, 1], F32, tag="srs")
            nc.vector.reciprocal(rsl[:], sd[:])
            u_s = psl.tile([r, D], BF16, tag="us")
            nc.vector.tensor_scalar(
                u_s[:], slots_ap[:], mv[:, 0:1], rsl[:], op0=Alu.subtract, op1=Alu.mult
            )
            usT = psl.tile([128, DC, r], BF16, tag="usT")
            for c in range(DC):
                tp = pp_w.tile([128, r], BF16, tag="a")
                nc.tensor.transpose(tp, u_s[:, c * 128 : (c + 1) * 128], id_bf[:r, :r])
                nc.vector.tensor_copy(usT[:, c, :], tp)

            qT = psl.tile([128, DC, r], BF16, tag="qT")
            for tj in range(DC):
                ps_q = pp_w.tile([128, r], F32, tag="b")
                for ci in range(DC):
                    nc.tensor.matmul(
                        ps_q, wd_s[:, ci, tj * 128 : (tj + 1) * 128], usT[:, ci, :],
                        start=(ci == 0), stop=(ci == DC - 1),
                    )
                nc.vector.tensor_scalar(
                    qT[:, tj, :], ps_q, qconst_c[:, tj : tj + 1], None, op0=Alu.add
                )
            if it == 0:
                tap("us", u_s[:], [r, D])
                tap("qT", qT[:], [128, DC * r])
            cstT = None
            if with_cst:
                cstT = psl.tile([1, r], BF16, tag="cstT")
                ps_c = pp_w.tile([1, r], F32, tag="b")
                for ci in range(DC):
                    nc.tensor.matmul(
                        ps_c, wqbk_s[:, ci : ci + 1], usT[:, ci, :],
                        start=(ci == 0), stop=False,
                    )
                nc.tensor.matmul(ps_c, cstc_bf, ones_r64_bf, start=False, stop=True)
                nc.vector.tensor_copy(cstT, ps_c)
            return qT, cstT

        def attn_softmax(b, qT, cstT, it):
            ps_d = pp_dots.tile([128, nt, S], F32, tag="dots")
            for t in range(nt):
                for c in range(DC):
                    nc.tensor.matmul(
                        ps_d[:, t, :],
                        y_t[:, c, b, t * 128 : (t + 1) * 128],
                        qT[:, c, b * S : (b + 1) * S],
                        start=(c == 0),
                        stop=(not with_cst and c == DC - 1),
                    )
                if with_cst:
                    nc.tensor.matmul(
                        ps_d[:, t, :], ones_r128_f8, cstT[:, b * S : (b + 1) * S],
                        start=False, stop=True,
                    )
            e_t = psm.tile([128, nt, S], F32, tag="et")
            nc.scalar.activation(e_t[:], ps_d[:], Act.Exp)
            ssum = psm.tile([128, nt], F32, tag="ssum")
            nc.vector.tensor_reduce(ssum[:], e_t[:], axis=mybir.AxisListType.X, op=Alu.add)
            rsum = psm.tile([128, nt], F32, tag="rsum")
            nc.vector.reciprocal(rsum[:], ssum[:])
            attn = psm.tile([128, nt, S], BF16, tag="attn", bufs=5)
            nc.vector.tensor_tensor(attn[:], e_t[:], _bcast_free(rsum[:], S), op=Alu.mult)
            if it == 0 and b == 0:
                tap("dots", ps_d[:], [128, nt * S])
                tap("attn", attn[:], [128, nt * S])
            return attn

        def ay_group(bs, attns, ayT, it):
            ps_ay = pp_ay.tile([128, D + 1], F32, tag="ay")
            for t in range(nt):
                for j, (b, attn) in enumerate(zip(bs, attns)):
                    nc.tensor.matmul(
                        ps_ay[32 * j : 32 * j + S, :], attn[:, t, :],
                        y_nat[:, b, t, :],
                        start=(t == 0), stop=(t == nt - 1),
                        tile_position=(0, 32 * j),
                    )
            for j, b in enumerate(bs):
                pslice = ps_ay[32 * j : 32 * j + S, :]
                den = psm.tile([S, 1], F32, tag="den")
                nc.vector.tensor_scalar(
                    den[:], pslice[:, D : D + 1], float(nl) * EPS, None, op0=Alu.add
                )
                rden = psm.tile([S, 1], F32, tag="rden")
                nc.vector.reciprocal(rden[:], den[:])
                ay_n = psm.tile([S, D], BF16, tag="ayn")
                nc.vector.tensor_scalar_mul(ay_n[:], pslice[:, :D], rden[:])
                if it == 0 and b == 0:
                    tap("ayraw", pslice[:], [S, D + 1])
                    tap("ayn", ay_n[:], [S, D])
                for c in range(DC):
                    tp = pp_w.tile([128, S], BF16, tag="a")
                    nc.tensor.transpose(
                        tp, ay_n[:, c * 128 : (c + 1) * 128], id_bf[:S, :S]
                    )
                    nc.vector.tensor_copy(ayT[:, c, b * S : (b + 1) * S], tp)

        def slot_phase(slots_ap, ayT, it):
            updT = psl.tile([128, DC, r], BF16, tag="updT")
            for to in range(DC):
                ps_u = pp_w.tile([128, r], F32, tag="b")
                for ci in range(DC):
                    nc.tensor.matmul(
                        ps_u, wvg_bf[:, ci, to * 128 : (to + 1) * 128], ayT[:, ci, :],
                        start=(ci == 0), stop=(ci == DC - 1),
                    )
                nc.vector.tensor_scalar(
                    updT[:, to, :], ps_u, bvpp_c[:, to : to + 1], None, op0=Alu.add
                )
            if it == 0:
                tap("updT", updT[:], [128, DC * r])

            slT = psl.tile([128, DC, r], BF16, tag="slT")
            sl_bf = psl.tile([r, D], BF16, tag="slbf")
            nc.vector.tensor_copy(sl_bf[:], slots_ap[:])
            for c in range(DC):
                tp = pp_w.tile([128, r], BF16, tag="a")
                nc.tensor.transpose(tp, sl_bf[:, c * 128 : (c + 1) * 128], id_bf[:r, :r])
                nc.vector.tensor_copy(slT[:, c, :], tp)

            gh_rz = pp_w.tile([r, 2 * D], F32, tag="a")
            gh_n = pp_w.tile([r, D], F32, tag="a")
            for (ps_g, lo, hi) in ((gh_rz, 0, 2 * D), (gh_n, 2 * D, 3 * D)):
                for ci in range(DC):
                    nc.tensor.matmul(
                        ps_g, slT[:, ci, :], whh_bf[:, ci, lo:hi],
                        start=(ci == 0), stop=False,
                    )
                nc.tensor.matmul(ps_g, ones_r64_bf, bhh_row[:, lo:hi], start=False, stop=True)
            ghs_rz = psl.tile([r, 2 * D], F32, tag="ghsrz")
            nc.scalar.copy(ghs_rz[:], gh_rz)
            ghs_n = psl.tile([r, D], F32, tag="ghsn")
            nc.scalar.copy(ghs_n[:], gh_n)

            gx_rz = pp_w.tile([r, 2 * D], F32, tag="b")
            gx_n = pp_w.tile([r, D], F32, tag="b")
            for (ps_g, lo, hi) in ((gx_rz, 0, 2 * D), (gx_n, 2 * D, 3 * D)):
                for ci in range(DC):
                    nc.tensor.matmul(
                        ps_g, updT[:, ci, :], wih_bf[:, ci, lo:hi],
                        start=(ci == 0), stop=False,
                    )
                nc.tensor.matmul(ps_g, ones_r64_bf, bih_row[:, lo:hi], start=False, stop=True)

            t_rz = psl.tile([r, 2 * D], F32, tag="trz")
            nc.vector.tensor_add(t_rz[:], gx_rz, ghs_rz[:])
            rz = psl.tile([r, 2 * D], F32, tag="rz")
            nc.scalar.activation(rz[:], t_rz[:], Act.Sigmoid)
            rhn = psl.tile([r, D], F32, tag="rhn")
            nc.vector.tensor_mul(rhn[:], rz[:, :D], ghs_n[:])
            nin = psl.tile([r, D], F32, tag="nin")
            nc.vector.tensor_add(nin[:], gx_n, rhn[:])
            n_t = psl.tile([r, D], F32, tag="nt")
            nc.scalar.activation(n_t[:], nin[:], Act.Tanh)
            d1 = psl.tile([r, D], F32, tag="d1")
            nc.vector.tensor_sub(d1[:], slots_ap[:], n_t[:])
            d2 = psl.tile([r, D], F32, tag="d2")
            nc.vector.tensor_mul(d2[:], rz[:, D : 2 * D], d1[:])
            snew = psl.tile([r, D], F32, tag="snew")
            nc.vector.tensor_add(snew[:], n_t[:], d2[:])
            if it == 0:
                tap("ghrz", ghs_rz[:], [r, 2 * D])
                tap("rz", rz[:], [r, 2 * D])
                tap("snew", snew[:], [r, D])

            st6b = psl.tile([r, 6], F32, tag="sst")
            nc.vector.bn_stats(st6b[:], snew[:])
            mvb = psl.tile([r, 2], F32, tag="smv")
            nc.vector.bn_aggr(mvb[:], st6b[:])
            sdb = psl.tile([r, 1], F32, tag="ssd")
            nc.scalar.activation(sdb[:], mvb[:, 1:2], Act.Sqrt, bias=lneps_t[:r])
            rslb = psl.tile([r, 1], F32, tag="srs")
            nc.vector.reciprocal(rslb[:], sdb[:])
            u_f = psl.tile([r, D], BF16, tag="uf")
            nc.vector.tensor_scalar(
                u_f[:], snew[:], mvb[:, 0:1], rslb[:], op0=Alu.subtract, op1=Alu.mult
            )
            ufT = psl.tile([128, DC, r], BF16, tag="ufT")
            for c in range(DC):
                tp = pp_w.tile([128, r], BF16, tag="a")
                nc.tensor.transpose(tp, u_f[:, c * 128 : (c + 1) * 128], id_bf[:r, :r])
                nc.vector.tensor_copy(ufT[:, c, :], tp)

            hT = psl.tile([128, HC, r], BF16, tag="hT")
            for to in range(HC):
                ps_h = pp_w.tile([128, r], F32, tag="b")
                for ci in range(DC):
                    nc.tensor.matmul(
                        ps_h, w1g_bf[:, ci, to * 128 : (to + 1) * 128], ufT[:, ci, :],
                        start=(ci == 0), stop=(ci == DC - 1),
                    )
                nc.scalar.activation(
                    hT[:, to, :], ps_h, Act.Relu, bias=mlpb_c[:, to : to + 1]
                )

            ps_o = pp_w.tile([r, D], F32, tag="b")
            for to in range(HC):
                nc.tensor.matmul(
                    ps_o, hT[:, to, :], w2_bf[:, to, :], start=(to == 0), stop=False
                )
            nc.tensor.matmul(ps_o, ones_r64_bf, b2_row, start=False, stop=True)

            slots_new = psl.tile([r, D], F32, tag="slots")
            nc.vector.tensor_add(slots_new[:], ps_o, snew[:])
            return slots_new

        # ---------------- input pass: LN + resident Y / Y^T ----------------
        y_nat = res.tile([128, bl, nt, D + 1], FP8)
        nc.vector.memset(y_nat[:, :, :, D : D + 1], 1.0)
        y_t = res.tile([128, DC, bl, nl], FP8)

        qT0, cstT0 = q_compute(slots, 0)
        ayT0 = psl.tile([128, DC, r], BF16, tag="ayT", bufs=2)
        attn0_tiles = []
        for b in range(bl):
            for t in range(nt):
                xin = pio.tile([128, D], F32)
                (nc.gpsimd if t % 2 == 0 else nc.sync).dma_start(
                    xin[:], x_d[b, t * 128 : (t + 1) * 128, :]
                )
                st6 = pln.tile([128, 6], F32)
                nc.vector.bn_stats(st6[:], xin[:])
                mv = pln.tile([128, 2], F32)
                nc.vector.bn_aggr(mv[:], st6[:])
                sd = pln.tile([128, 1], F32)
                nc.scalar.activation(sd[:], mv[:, 1:2], Act.Sqrt, bias=lneps_t[:])
                rs_ = pln.tile([128, 1], F32)
                nc.vector.reciprocal(rs_[:], sd[:])
                xln = pln.tile([128, D], BF16)
                nc.vector.tensor_scalar(
                    xln[:], xin[:], mv[:, 0:1], rs_[:], op0=Alu.subtract, op1=Alu.mult
                )
                nc.scalar.copy(y_nat[:, b, t, :D], xln[:])
                if b == 0 and t == 0:
                    tap("y00", y_nat[:, 0, 0, :D], [128, D])
                for c in range(DC):
                    yps = pp_w.tile([128, 128], BF16, tag="a" if c == 0 else "b")
                    nc.tensor.transpose(yps, xln[:, c * 128 : (c + 1) * 128], id_bf)
                    if (t + c) % 2 == 0:
                        nc.vector.tensor_copy(
                            y_t[:, c, b, t * 128 : (t + 1) * 128], yps
                        )
                    else:
                        nc.scalar.copy(y_t[:, c, b, t * 128 : (t + 1) * 128], yps)
                    if b == 0 and t == 0:
                        tap(f"yt0{c}", y_t[:, c, 0, 0:128], [128, 128])
            attn0_tiles.append(attn_softmax(b, qT0, cstT0, 0))
            if b % 4 == 3:
                ay_group(list(range(b - 3, b + 1)), attn0_tiles[-4:], ayT0, 0)

        # ---------------- iterations (iter-0 attention overlapped above) ----------------
        slots = slot_phase(slots, ayT0, 0)
        for it in range(1, iters):
            qT_i, cstT_i = q_compute(slots, it)
            ayT_i = psl.tile([128, DC, r], BF16, tag="ayT", bufs=2)
            atiles = []
            for b in range(bl):
                atiles.append(attn_softmax(b, qT_i, cstT_i, it))
                if b % 4 == 3:
                    ay_group(list(range(b - 3, b + 1)), atiles[-4:], ayT_i, it)
            slots = slot_phase(slots, ayT_i, it)

        nc.gpsimd.dma_start(out_d, slots[:])

    nc.compile()
    return nc


_CACHE = {}


def _get_nc():
    if "nc" not in _CACHE:
        _CACHE["nc"] = build()
    return _CACHE["nc"]


def kernel(inputs, slots, wq, bq, wk, bk, wv, bv, w_ih, b_ih, w_hh, b_hh,
           mlp_w1, mlp_b1, mlp_w2, mlp_b2, g_in, b_in, g_sl, b_sl, g_ff, b_ff,
           _trace=False):
    nc = _get_nc()
    inputs = np.ascontiguousarray(np.asarray(inputs, np.float32))
    slots = np.asarray(slots, np.float32)
    shared = dict(
        wq=np.asarray(wq, np.float32), bq=np.asarray(bq, np.float32),
        wk=np.asarray(wk, np.float32), bk=np.asarray(bk, np.float32),
        wv=np.asarray(wv, np.float32), bv=np.asarray(bv, np.float32),
        w_ih=np.asarray(w_ih, np.float32), b_ih=np.asarray(b_ih, np.float32),
        w_hh=np.asarray(w_hh, np.float32), b_hh=np.asarray(b_hh, np.float32),
        mlp_w1=np.asarray(mlp_w1, np.float32), mlp_b1=np.asarray(mlp_b1, np.float32),
        mlp_w2=np.asarray(mlp_w2, np.float32), mlp_b2=np.asarray(mlp_b2, np.float32),
        g_in=np.asarray(g_in, np.float32), b_in=np.asarray(b_in, np.float32),
        g_sl=np.asarray(g_sl, np.float32), b_sl=np.asarray(b_sl, np.float32),
        g_ff=np.asarray(g_ff, np.float32), b_ff=np.asarray(b_ff, np.float32),
    )
    in_maps = []
    for c in range(NCORES):
        bsl = slice(c * BL, (c + 1) * BL)
        m = dict(shared)
        m["x"] = inputs[bsl]
        m["slots0"] = np.ascontiguousarray(
            slots[:, bsl, :].transpose(1, 0, 2).reshape(R, D)
        )
        in_maps.append(m)

    res = run_bass_kernel_spmd(nc, in_maps, list(range(NCORES)), trace=_trace)
    out = np.empty((S, B, D), np.float32)
    for c in range(NCORES):
        r = np.asarray(res.results[c]["out"]).reshape(BL, S, D)
        out[:, c * BL : (c + 1) * BL, :] = r.transpose(1, 0, 2)
    if _trace:
        _CACHE["last_results"] = res
    return out
